# revision 49
# baseline (speedup 1.0000x reference)
"""Multi-head attention Bass/Tile kernel for 8 TRN2 NeuronCores.

Problem: nn_MultiHeadAttention (B=4, T1=T2=2048, d_model=256, d_key=32, H=8,
per-head value dim = d_model).  Reference math (no score scaling, no mask):

    k = key   @ WK^T + bk           [B, T1, 256]   (head h -> cols 32h..32h+32)
    q = query @ WQ^T + bq           [B, T2, 256]
    v = value @ WV^T + bv           [B, T1, 2048]  (head h -> cols 256h..256h+256)
    scores_h = k_h q_h^T            [T1, T2]
    attn = softmax over T1 (keys)
    emb_h = attn^T v_h              [T2, 256]
    out = emb' @ WO^T + bo          emb' channel c = d*8 + h (d outer, h inner)

Sharding: core c handles (batch b = c//2, query half qs = c%2) -> each core
computes the full output slice out[b, qs*1024:(qs+1)*1024, :].  No collectives.

Algebraic restructure (all matmuls bf16, fp32 PSUM):  WV and WO are folded
into per-head G_h[m,o] = sum_d WV[h*256+d, m] WO[o, d*8+h], so the value path
is U_h = val @ G_h (one [2048,256] tensor per head) and the output is
out[q,:] = sum_h (E_h^T U'_h)[q,:]/denom_h[q] + bias, where E = exp(scores),
U' = [U | ones] so PSUM column 256 of the E^T U' matmul IS the softmax
denominator (TRN2 matmul cost scales only with the moving-operand free dim,
so the extra column is free), and bias[o] = wob[o] + sum_h sum_d wvb[h*256+d]
WO[o, d*8+h] (softmax rows sum to 1, so the v-bias is a constant).

Host-side prep (free): everything is cast to bf16 and packed into exactly
TWO dram tensors -- kqv_x = [key; qry; val; WK; WQ; bias rows] feeds one XBAR
DMA-transpose that lands every m-major operand (weight ROWS transpose into
W^T columns, bias rows land as per-partition scalars), and wvo = [WV; WO
head-outer-permuted; v/o bias rows] is one linear DMA.  Per-DMA issue
overhead is ~2.7us and same-queue DMAs serialize, so DMA COUNT, not bytes,
sets the startup latency.  The device does zero layout work on PE/ACT.

The main loop is software-pipelined: scores+exp of iteration i+1 are emitted
before the E^T U' chains of iteration i, so the PE streams scores while ACT
finishes the exps that the E^T U' chains depend on.

kernel(**inputs) takes the FULL unsharded inputs and returns the full output.
"""

import numpy as np
import ml_dtypes
from contextlib import ExitStack

import concourse.bass as bass
import concourse.bacc as bacc
import concourse.mybir as mybir
import concourse.tile as tile
from concourse.bass_utils import run_bass_kernel_spmd

P = 128
B, T1, T2, DM, DK, H = 4, 2048, 2048, 256, 32, 8
QSH = T2 // 2  # queries per core
N_CORES = 8

F32 = mybir.dt.float32
BF16 = mybir.dt.bfloat16
AF = mybir.ActivationFunctionType

ST = T1 // P        # 16 key/seq tiles
QT = QSH // P       # 8 query tiles per core
QC = 512            # query chunk (PSUM free dim)
NQC = QSH // QC     # 2 query chunks
UO = DM + 1         # U columns incl. the ones column (denominator)


def _build_bass():
    nc = bacc.Bacc("TRN2", target_bir_lowering=False, debug=False)

    # kqv = [key; qry; val; WK; WQ] -- one XBAR transpose feeds the whole
    # k/q/v path in m-major layout (weight rows transpose to W^T columns)
    kqv = nc.dram_tensor("kqv_x", [2 * T1 + QSH + 2 * DM + 16, DM], BF16,
                         kind="ExternalInput").ap()
    wvo = nc.dram_tensor("wvo", [2 * H * DM + 2 * P, DM], BF16,
                         kind="ExternalInput").ap()
    out = nc.dram_tensor("out_y", [QSH, DM], F32, kind="ExternalOutput").ap()

    with tile.TileContext(nc, pool_alloc_mode="queue") as tc:
        with ExitStack() as ctx:
            _body(ctx, tc, kqv, wvo, out)
    nc.compile()
    return nc


def _body(ctx, tc, kqv, wvo, out):
    nc = tc.nc
    mult, add = mybir.AluOpType.mult, mybir.AluOpType.add
    consts = ctx.enter_context(tc.tile_pool(name="consts", bufs=1))
    main = ctx.enter_context(tc.tile_pool(name="main", bufs=1))
    # One PSUM pool, 3 tags / 8 banks total:
    #   tag S: 2 banks x2      (score tiles [128,2,512] f32)
    #   tag P: 1 bank  x2      (E^T U' output tiles [128,257] f32; bias-const)
    #   tag U: 1 bank  x2      (k/q/U/G projection tiles; warmup)
    pP = ctx.enter_context(tc.tile_pool(name="pP", bufs=1, space="PSUM"))

    bias_bc = consts.tile([P, DM], F32)   # broadcast final bias (filled later)

    # PE warmup: ~4us of throwaway matmuls on a zeroed tile, overlapping the
    # initial DMAs, so the p-state ramp is done before real matmuls start.
    warm = consts.tile([P, QC], BF16)
    nc.vector.memset(warm, 0.0)
    for i in range(44):
        pw = pP.tile([P, QC], F32, tag="U", name=f"warm{i}", bufs=2)
        nc.tensor.matmul(pw, warm[:, 0:P], warm, start=True, stop=True)

    # persistent bf16 tensors
    kT = main.tile([P, 2, T1], BF16)      # [c, s]
    qT = main.tile([P, 2, QSH], BF16)     # [c, q]
    kqvT = main.tile([P, 2, 2 * T1 + QSH + 2 * DM + 16], BF16)
    Gt = main.tile([P, 2, H, DM], BF16)   # [m, mt, h, o]
    uT = main.tile([P, 2, ST, UO], BF16)  # [s, hslot, st, o]; col 256 = 1.0
    acc = main.tile([P, QT, DM], F32)     # output accumulator [q, cout]
    nc.vector.memset(uT[:, :, :, DM:UO], 1.0)

    # ---------------- stage 0: DMA loads/transposes + projections -----------
    with ExitStack() as s0:
        stg = s0.enter_context(tc.tile_pool(name="stg", bufs=1))

        # Minimal DMA count: per-DMA issue overhead is ~2.7us and queue DMAs
        # serialize, so key/qry/val ride ONE stacked XBAR transpose.
        nc.sync.dma_start_transpose(kqvT, kqv)
        wvo_bf = stg.tile([P, 2 * ST + 2, DM], BF16)
        nc.sync.dma_start(out=wvo_bf, in_=wvo.rearrange("(t p) d -> p t d", p=P))
        nb = 2 * T1 + QSH + 2 * DM
        wk_b, wq_b = kqvT[:, :, nb:nb + 1], kqvT[:, :, nb + 1:nb + 2]
        wvb_bf = wvo_bf[:, 2 * ST, 4:4 + ST]
        wob_f = wvo_bf[0:1, 2 * ST + 1, :]
        keyT = kqvT[:, :, 0:T1]               # [m, s]
        qryT = kqvT[:, :, T1:T1 + QSH]        # [m, q]
        valT = kqvT[:, :, T1 + QSH:2 * T1 + QSH]  # [m, s]
        wkT = kqvT[:, :, 2 * T1 + QSH:2 * T1 + QSH + DM]      # [m, c]
        wqT = kqvT[:, :, 2 * T1 + QSH + DM:2 * T1 + QSH + 2 * DM]
        wv_bf = wvo_bf[:, 0:ST, :]            # [c_v, kt, m] (natural)
        woTp = wvo_bf[:, ST:2 * ST, :]        # [d (in-head), kt=2h+db, o]

        # k/q projections: kT[c, s] = sum_m wkT[m, c] keyT[m, s]  (+bias)
        for ct in range(2):
            for sc in range(T1 // 512):
                pp = pP.tile([P, 512], F32, tag="U", name=f"ppk{ct}_{sc}", bufs=2)
                for dt in range(2):
                    nc.tensor.matmul(pp, wkT[:, dt, ct * P:(ct + 1) * P],
                                     keyT[:, dt, sc * 512:(sc + 1) * 512],
                                     start=(dt == 0), stop=(dt == 1))
                nc.scalar.activation(out=kT[:, ct, sc * 512:(sc + 1) * 512], in_=pp,
                                     func=AF.Identity, bias=wk_b[:, ct, :])
            for sc in range(QSH // 512):
                pp = pP.tile([P, 512], F32, tag="U", name=f"ppq{ct}_{sc}", bufs=2)
                for dt in range(2):
                    nc.tensor.matmul(pp, wqT[:, dt, ct * P:(ct + 1) * P],
                                     qryT[:, dt, sc * 512:(sc + 1) * 512],
                                     start=(dt == 0), stop=(dt == 1))
                nc.scalar.activation(out=qT[:, ct, sc * 512:(sc + 1) * 512], in_=pp,
                                     func=AF.Identity, bias=wq_b[:, ct, :])

        # G_h[m, o] = sum_d WV[h*256+d, m] WO[o, d*8+h]  (WV/WO folded)
        for h in range(H):
            pg = pP.tile([P, 2, DM], F32, tag="U", name=f"pg{h}", bufs=2)
            for mt in range(2):
                for db in range(2):
                    nc.tensor.matmul(pg[:, mt, :],
                                     wv_bf[:, 2 * h + db, mt * P:(mt + 1) * P],
                                     woTp[:, 2 * h + db, :],
                                     start=(db == 0), stop=(db == 1))
            nc.vector.tensor_copy(out=Gt[:, :, h, :], in_=pg)

        # bias_bc[o] = wob[o] + sum_h sum_d wvb[h*256+d] WO[o, d*8+h]
        pb = pP.tile([1, DM], F32, tag="P", name="pbias", bufs=2)
        for kt in range(ST):
            nc.tensor.matmul(pb, wvb_bf[:, kt:kt + 1], woTp[:, kt, :],
                             start=(kt == 0), stop=(kt == ST - 1))
        bias1 = consts.tile([1, DM], F32)
        nc.vector.tensor_add(bias1, pb, wob_f)
        nc.gpsimd.partition_broadcast(bias_bc, bias1)

    # ---------------- main loop: one head at a time, software-pipelined -----
    with ExitStack() as sm:
        sE = sm.enter_context(tc.tile_pool(name="sE", bufs=2))
        ssm = sm.enter_context(tc.tile_pool(name="ssm", bufs=4))

        out_r = out.rearrange("(n p) d -> p n d", p=P)

        def emit_po(h, qc, E):
            """out_h[q, :] = E^T U' (col 256 = denominator), normalize, acc.
            On the last head, stream each finished acc tile straight out."""
            hs = h % 2
            for qt in range(QC // P):
                po = pP.tile([P, UO], F32, tag="P",
                             name=f"po{h}_{qc}_{qt}", bufs=2)
                for st in range(ST):
                    nc.tensor.matmul(po, E[:, st, qt * P:(qt + 1) * P],
                                     uT[:, hs, st, :],
                                     start=(st == 0), stop=(st == ST - 1))
                rc = ssm.tile([P, 1], F32, tag="rc", name=f"rc{h}_{qc}_{qt}")
                nc.vector.reciprocal(out=rc, in_=po[:, DM:UO])
                gqt = qc * (QC // P) + qt
                nc.vector.scalar_tensor_tensor(
                    out=acc[:, gqt, :], in0=po[:, 0:DM], scalar=rc,
                    in1=(bias_bc if h == 0 else acc[:, gqt, :]),
                    op0=mult, op1=add)
                if h == H - 1 and qt % 2 == 1:
                    g0 = qc * 4 + qt - 1
                    nc.sync.dma_start(out=out_r[:, g0:g0 + 2, :],
                                      in_=acc[:, g0:g0 + 2, :])

        def emit_u(h):
            """U_h[s, o] = sum_m val[s, m] G_h[m, o]; col 256 stays 1.0."""
            hs = h % 2
            for sp in range(ST // 2):
                pu = pP.tile([P, 2, DM], F32, tag="U", name=f"pu{h}_{sp}", bufs=2)
                for i in range(2):
                    st = 2 * sp + i
                    for mt in range(2):
                        nc.tensor.matmul(pu[:, i, :],
                                         valT[:, mt, st * P:(st + 1) * P],
                                         Gt[:, mt, h, :],
                                         start=(mt == 0), stop=(mt == 1))
                nc.vector.tensor_copy(out=uT[:, hs, 2 * sp:2 * sp + 2, 0:DM],
                                      in_=pu)

        prev = None
        for h in range(H):
            emit_u(h)
            base, ctile = 32 * (h % 4), h // 4
            for qc in range(NQC):
                E = sE.tile([P, ST, QC], BF16, tag="E", name=f"E{h}_{qc}")
                # phase 1: scores + exp.  scores_h[s, q] = kT_h^T qT_h
                for sp in range(ST // 2):
                    ps = pP.tile([P, 2, QC], F32, tag="S",
                                 name=f"sc{h}_{qc}_{sp}", bufs=2)
                    for i in range(2):
                        st = 2 * sp + i
                        nc.tensor.matmul(
                            ps[:, i, :],
                            kT[base:base + 32, ctile, st * P:(st + 1) * P],
                            qT[base:base + 32, ctile, qc * QC:(qc + 1) * QC],
                            start=True, stop=True, tile_position=(base, 0))
                    nc.scalar.activation(out=E[:, 2 * sp:2 * sp + 2, :], in_=ps,
                                         func=AF.Exp)
                if prev is not None:
                    emit_po(*prev)
                prev = (h, qc, E)
        emit_po(*prev)


_NC_CACHE = None


def _get_nc():
    global _NC_CACHE
    if _NC_CACHE is None:
        _NC_CACHE = _build_bass()
    return _NC_CACHE


def _bf(x):
    return np.ascontiguousarray(np.asarray(x, dtype=np.float32).astype(
        ml_dtypes.bfloat16))


def _make_in_maps(inputs):
    wo = np.asarray(inputs["WO_w"], dtype=np.float32)     # [256, 2048]
    # woTp row (2h+db)*128+d' = WO[:, (db*128+d')*8+h]
    wotp = wo.reshape(DM, 2, P, H).transpose(3, 1, 2, 0).reshape(H * DM, DM)
    wvo_h = np.concatenate([np.asarray(inputs["WV_w"], dtype=np.float32), wotp])
    # wvo tail rows: row +0 cols 4:20 = WV_b (column kt = partition slice of
    # it), row +1 = WO_b
    extra = np.zeros((2 * P, DM), dtype=np.float32)
    extra[0:P, 4:4 + ST] = np.asarray(
        inputs["WV_b"], dtype=np.float32).reshape(ST, P).T
    extra[P, :] = np.asarray(inputs["WO_b"], dtype=np.float32)
    # kqv tail: WK/WQ rows (transpose to W^T), then 16 pad rows whose first
    # two are WK_b/WQ_b (a transposed bias row lands as [p, tile] scalars)
    wkq_n = np.concatenate([np.asarray(inputs["WK_w"], dtype=np.float32),
                            np.asarray(inputs["WQ_w"], dtype=np.float32)])
    kqb_rows = np.zeros((16, DM), dtype=np.float32)
    kqb_rows[0] = np.asarray(inputs["WK_b"], dtype=np.float32)
    kqb_rows[1] = np.asarray(inputs["WQ_b"], dtype=np.float32)
    kqv_tail = _bf(np.concatenate([wkq_n, kqb_rows]))
    shared = {
        "wvo": _bf(np.concatenate([wvo_h, extra])),
    }
    key_in = _bf(inputs["key_input"])
    qry_in = _bf(inputs["query_input"])
    val_in = _bf(inputs["value_input"])
    in_maps = []
    for c in range(N_CORES):
        b, qs = c // 2, c % 2
        in_maps.append(dict(
            shared,
            kqv_x=np.ascontiguousarray(np.concatenate([
                key_in[b], qry_in[b, qs * QSH:(qs + 1) * QSH], val_in[b],
                kqv_tail])),
        ))
    return in_maps


def _assemble(results):
    out = np.empty((B, T2, DM), dtype=np.float32)
    for c in range(N_CORES):
        b, qs = c // 2, c % 2
        out[b, qs * QSH:(qs + 1) * QSH] = results[c]["out_y"]
    return out


def run_spmd(inputs, **kwargs):
    """Run the kernel on all 8 cores; kwargs forwarded (e.g. trace=True)."""
    nc = _get_nc()
    res = run_bass_kernel_spmd(nc, _make_in_maps(inputs),
                               core_ids=list(range(N_CORES)), **kwargs)
    return res


def kernel(**inputs):
    res = run_spmd(inputs)
    return _assemble(res.results)


# revision 56
# speedup vs baseline: 1.0162x; 1.0162x over previous
"""Multi-head attention Bass/Tile kernel for 8 TRN2 NeuronCores.

Problem: nn_MultiHeadAttention (B=4, T1=T2=2048, d_model=256, d_key=32, H=8,
per-head value dim = d_model).  Reference math (no score scaling, no mask):

    k = key   @ WK^T + bk           [B, T1, 256]   (head h -> cols 32h..32h+32)
    q = query @ WQ^T + bq           [B, T2, 256]
    v = value @ WV^T + bv           [B, T1, 2048]  (head h -> cols 256h..256h+256)
    scores_h = k_h q_h^T            [T1, T2]
    attn = softmax over T1 (keys)
    emb_h = attn^T v_h              [T2, 256]
    out = emb' @ WO^T + bo          emb' channel c = d*8 + h (d outer, h inner)

Sharding: core c handles (batch b = c//2, query half qs = c%2) -> each core
computes the full output slice out[b, qs*1024:(qs+1)*1024, :].  No collectives.

Algebraic restructure (all matmuls bf16, fp32 PSUM):  WV and WO are folded
into per-head G_h[m,o] = sum_d WV[h*256+d, m] WO[o, d*8+h], so the value path
is U_h = val @ G_h (one [2048,256] tensor per head) and the output is
out[q,:] = sum_h (E_h^T U'_h)[q,:]/denom_h[q] + bias, where E = exp(scores),
U' = [U | ones] so PSUM column 256 of the E^T U' matmul IS the softmax
denominator (TRN2 matmul cost scales only with the moving-operand free dim,
so the extra column is free), and bias[o] = wob[o] + sum_h sum_d wvb[h*256+d]
WO[o, d*8+h] (softmax rows sum to 1, so the v-bias is a constant).

Host-side prep (free): everything is cast to bf16 and packed into exactly
TWO dram tensors -- kqv_x = [key; qry; val; WK; WQ; bias rows] feeds one XBAR
DMA-transpose that lands every m-major operand (weight ROWS transpose into
W^T columns, bias rows land as per-partition scalars), and wvo = [WV; WO
head-outer-permuted; v/o bias rows] is one linear DMA.  Per-DMA issue
overhead is ~2.7us and same-queue DMAs serialize, so DMA COUNT, not bytes,
sets the startup latency.  The device does zero layout work on PE/ACT.

The main loop is software-pipelined: scores+exp of iteration i+1 are emitted
before the E^T U' chains of iteration i, so the PE streams scores while ACT
finishes the exps that the E^T U' chains depend on.

kernel(**inputs) takes the FULL unsharded inputs and returns the full output.
"""

import numpy as np
import ml_dtypes
from contextlib import ExitStack

import concourse.bass as bass
import concourse.bacc as bacc
import concourse.mybir as mybir
import concourse.tile as tile
from concourse.bass_utils import run_bass_kernel_spmd

P = 128
B, T1, T2, DM, DK, H = 4, 2048, 2048, 256, 32, 8
QSH = T2 // 2  # queries per core
N_CORES = 8

F32 = mybir.dt.float32
BF16 = mybir.dt.bfloat16
AF = mybir.ActivationFunctionType

ST = T1 // P        # 16 key/seq tiles
QT = QSH // P       # 8 query tiles per core
QC = 512            # query chunk (PSUM free dim)
NQC = QSH // QC     # 2 query chunks
UO = DM + 1         # U columns incl. the ones column (denominator)


def _build_bass():
    nc = bacc.Bacc("TRN2", target_bir_lowering=False, debug=False)

    # kqv = [key; qry; val; WK; WQ] -- one XBAR transpose feeds the whole
    # k/q/v path in m-major layout (weight rows transpose to W^T columns)
    kqv = nc.dram_tensor("kqv_x", [T1 + QSH + 2 * DM + 16, DM], BF16,
                         kind="ExternalInput").ap()
    vli = nc.dram_tensor("vli_x", [T1, DM], BF16, kind="ExternalInput").ap()
    wvo = nc.dram_tensor("wvo", [2 * H * DM + 2 * P, DM], BF16,
                         kind="ExternalInput").ap()
    out = nc.dram_tensor("out_y", [QSH, DM], F32, kind="ExternalOutput").ap()

    with tile.TileContext(nc, pool_alloc_mode="queue") as tc:
        with ExitStack() as ctx:
            _body(ctx, tc, kqv, vli, wvo, out)
    nc.compile()
    return nc


def _body(ctx, tc, kqv, vli, wvo, out):
    nc = tc.nc
    mult, add = mybir.AluOpType.mult, mybir.AluOpType.add
    consts = ctx.enter_context(tc.tile_pool(name="consts", bufs=1))
    main = ctx.enter_context(tc.tile_pool(name="main", bufs=1))
    # One PSUM pool, 3 tags / 8 banks total:
    #   tag S: 2 banks x2      (score tiles [128,2,512] f32)
    #   tag P: 1 bank  x2      (E^T U' output tiles [128,257] f32; bias-const)
    #   tag U: 1 bank  x2      (k/q/U/G projection tiles; warmup)
    pP = ctx.enter_context(tc.tile_pool(name="pP", bufs=1, space="PSUM"))

    bias_bc = consts.tile([P, DM], F32)   # broadcast final bias (filled later)

    # PE warmup: throwaway matmuls on a zeroed tile, overlapping the
    # initial DMAs, so the p-state ramp is done before real matmuls start.
    warm = consts.tile([P, QC], BF16)
    nc.vector.memset(warm, 0.0)
    for i in range(20):
        pw = pP.tile([P, QC], F32, tag="U", name=f"warm{i}", bufs=2)
        nc.tensor.matmul(pw, warm[:, 0:P], warm, start=True, stop=True)

    # persistent bf16 tensors
    kT = main.tile([P, 2, T1], BF16)      # [c, s]
    qT = main.tile([P, 2, QSH], BF16)     # [c, q]
    kqvT = main.tile([P, 2, T1 + QSH + 2 * DM + 16], BF16)
    valT = main.tile([P, 2, T1], BF16)
    Gt = main.tile([P, 2, H, DM], BF16)   # [m, mt, h, o]
    uT = main.tile([P, 2, ST, UO], BF16)  # [s, hslot, st, o]; col 256 = 1.0
    acc = main.tile([P, QT, DM], F32)     # output accumulator [q, cout]
    nc.vector.memset(uT[:, :, :, DM:UO], 1.0)

    # ---------------- stage 0: DMA loads/transposes + projections -----------
    with ExitStack() as s0:
        stg = s0.enter_context(tc.tile_pool(name="stg", bufs=1))

        # Minimal DMA count: per-DMA issue overhead is ~2.7us and queue DMAs
        # serialize, so key/qry/val ride ONE stacked XBAR transpose.
        nc.sync.dma_start_transpose(kqvT, kqv)
        wvo_bf = stg.tile([P, 2 * ST + 2, DM], BF16)
        nc.sync.dma_start(out=wvo_bf, in_=wvo.rearrange("(t p) d -> p t d", p=P))
        nc.sync.dma_start_transpose(valT, vli)
        nb = T1 + QSH + 2 * DM
        wk_b, wq_b = kqvT[:, :, nb:nb + 1], kqvT[:, :, nb + 1:nb + 2]
        wvb_bf = wvo_bf[:, 2 * ST, 4:4 + ST]
        wob_f = wvo_bf[0:1, 2 * ST + 1, :]
        keyT = kqvT[:, :, 0:T1]               # [m, s]
        qryT = kqvT[:, :, T1:T1 + QSH]        # [m, q]
        wkT = kqvT[:, :, T1 + QSH:T1 + QSH + DM]              # [m, c]
        wqT = kqvT[:, :, T1 + QSH + DM:T1 + QSH + 2 * DM]
        wv_bf = wvo_bf[:, 0:ST, :]            # [c_v, kt, m] (natural)
        woTp = wvo_bf[:, ST:2 * ST, :]        # [d (in-head), kt=2h+db, o]

        # k/q projections: kT[c, s] = sum_m wkT[m, c] keyT[m, s]  (+bias)
        for ct in range(2):
            for sc in range(T1 // 512):
                pp = pP.tile([P, 512], F32, tag="U", name=f"ppk{ct}_{sc}", bufs=2)
                for dt in range(2):
                    nc.tensor.matmul(pp, wkT[:, dt, ct * P:(ct + 1) * P],
                                     keyT[:, dt, sc * 512:(sc + 1) * 512],
                                     start=(dt == 0), stop=(dt == 1))
                nc.scalar.activation(out=kT[:, ct, sc * 512:(sc + 1) * 512], in_=pp,
                                     func=AF.Identity, bias=wk_b[:, ct, :])
            for sc in range(QSH // 512):
                pp = pP.tile([P, 512], F32, tag="U", name=f"ppq{ct}_{sc}", bufs=2)
                for dt in range(2):
                    nc.tensor.matmul(pp, wqT[:, dt, ct * P:(ct + 1) * P],
                                     qryT[:, dt, sc * 512:(sc + 1) * 512],
                                     start=(dt == 0), stop=(dt == 1))
                nc.scalar.activation(out=qT[:, ct, sc * 512:(sc + 1) * 512], in_=pp,
                                     func=AF.Identity, bias=wq_b[:, ct, :])

        # G_h[m, o] = sum_d WV[h*256+d, m] WO[o, d*8+h]  (WV/WO folded)
        for h in range(H):
            pg = pP.tile([P, 2, DM], F32, tag="U", name=f"pg{h}", bufs=2)
            for mt in range(2):
                for db in range(2):
                    nc.tensor.matmul(pg[:, mt, :],
                                     wv_bf[:, 2 * h + db, mt * P:(mt + 1) * P],
                                     woTp[:, 2 * h + db, :],
                                     start=(db == 0), stop=(db == 1))
            nc.vector.tensor_copy(out=Gt[:, :, h, :], in_=pg)

        # bias_bc[o] = wob[o] + sum_h sum_d wvb[h*256+d] WO[o, d*8+h]
        pb = pP.tile([1, DM], F32, tag="P", name="pbias", bufs=2)
        for kt in range(ST):
            nc.tensor.matmul(pb, wvb_bf[:, kt:kt + 1], woTp[:, kt, :],
                             start=(kt == 0), stop=(kt == ST - 1))
        bias1 = consts.tile([1, DM], F32)
        nc.vector.tensor_add(bias1, pb, wob_f)
        nc.gpsimd.partition_broadcast(bias_bc, bias1)

    # ---------------- main loop: one head at a time, software-pipelined -----
    with ExitStack() as sm:
        sE = sm.enter_context(tc.tile_pool(name="sE", bufs=2))
        ssm = sm.enter_context(tc.tile_pool(name="ssm", bufs=4))

        out_r = out.rearrange("(n p) d -> p n d", p=P)

        def emit_po(h, qc, E):
            """out_h[q, :] = E^T U' (col 256 = denominator), normalize, acc.
            On the last head, stream each finished acc tile straight out."""
            hs = h % 2
            for qt in range(QC // P):
                po = pP.tile([P, UO], F32, tag="P",
                             name=f"po{h}_{qc}_{qt}", bufs=2)
                for st in range(ST):
                    nc.tensor.matmul(po, E[:, st, qt * P:(qt + 1) * P],
                                     uT[:, hs, st, :],
                                     start=(st == 0), stop=(st == ST - 1))
                rc = ssm.tile([P, 1], F32, tag="rc", name=f"rc{h}_{qc}_{qt}")
                nc.vector.reciprocal(out=rc, in_=po[:, DM:UO])
                gqt = qc * (QC // P) + qt
                nc.vector.scalar_tensor_tensor(
                    out=acc[:, gqt, :], in0=po[:, 0:DM], scalar=rc,
                    in1=(bias_bc if h == 0 else acc[:, gqt, :]),
                    op0=mult, op1=add)
                if h == H - 1 and qt % 2 == 1:
                    g0 = qc * 4 + qt - 1
                    nc.sync.dma_start(out=out_r[:, g0:g0 + 2, :],
                                      in_=acc[:, g0:g0 + 2, :])

        def emit_u(h):
            """U_h[s, o] = sum_m val[s, m] G_h[m, o]; col 256 stays 1.0."""
            hs = h % 2
            for sp in range(ST // 2):
                pu = pP.tile([P, 2, DM], F32, tag="U", name=f"pu{h}_{sp}", bufs=2)
                for i in range(2):
                    st = 2 * sp + i
                    for mt in range(2):
                        nc.tensor.matmul(pu[:, i, :],
                                         valT[:, mt, st * P:(st + 1) * P],
                                         Gt[:, mt, h, :],
                                         start=(mt == 0), stop=(mt == 1))
                nc.vector.tensor_copy(out=uT[:, hs, 2 * sp:2 * sp + 2, 0:DM],
                                      in_=pu)

        prev = None
        for h in range(H):
            if h > 0:
                emit_u(h)
            base, ctile = 32 * (h % 4), h // 4
            for qc in range(NQC):
                E = sE.tile([P, ST, QC], BF16, tag="E", name=f"E{h}_{qc}")
                # phase 1: scores + exp.  scores_h[s, q] = kT_h^T qT_h
                for sp in range(ST // 2):
                    ps = pP.tile([P, 2, QC], F32, tag="S",
                                 name=f"sc{h}_{qc}_{sp}", bufs=2)
                    for i in range(2):
                        st = 2 * sp + i
                        nc.tensor.matmul(
                            ps[:, i, :],
                            kT[base:base + 32, ctile, st * P:(st + 1) * P],
                            qT[base:base + 32, ctile, qc * QC:(qc + 1) * QC],
                            start=True, stop=True, tile_position=(base, 0))
                    nc.scalar.activation(out=E[:, 2 * sp:2 * sp + 2, :], in_=ps,
                                         func=AF.Exp)
                if h == 0 and qc == 0:
                    emit_u(0)
                if prev is not None:
                    emit_po(*prev)
                prev = (h, qc, E)
        emit_po(*prev)


_NC_CACHE = None


def _get_nc():
    global _NC_CACHE
    if _NC_CACHE is None:
        _NC_CACHE = _build_bass()
    return _NC_CACHE


def _bf(x):
    return np.ascontiguousarray(np.asarray(x, dtype=np.float32).astype(
        ml_dtypes.bfloat16))


def _make_in_maps(inputs):
    wo = np.asarray(inputs["WO_w"], dtype=np.float32)     # [256, 2048]
    # woTp row (2h+db)*128+d' = WO[:, (db*128+d')*8+h]
    wotp = wo.reshape(DM, 2, P, H).transpose(3, 1, 2, 0).reshape(H * DM, DM)
    wvo_h = np.concatenate([np.asarray(inputs["WV_w"], dtype=np.float32), wotp])
    # wvo tail rows: row +0 cols 4:20 = WV_b (column kt = partition slice of
    # it), row +1 = WO_b
    extra = np.zeros((2 * P, DM), dtype=np.float32)
    extra[0:P, 4:4 + ST] = np.asarray(
        inputs["WV_b"], dtype=np.float32).reshape(ST, P).T
    extra[P, :] = np.asarray(inputs["WO_b"], dtype=np.float32)
    # kqv tail: WK/WQ rows (transpose to W^T), then 16 pad rows whose first
    # two are WK_b/WQ_b (a transposed bias row lands as [p, tile] scalars)
    wkq_n = np.concatenate([np.asarray(inputs["WK_w"], dtype=np.float32),
                            np.asarray(inputs["WQ_w"], dtype=np.float32)])
    kqb_rows = np.zeros((16, DM), dtype=np.float32)
    kqb_rows[0] = np.asarray(inputs["WK_b"], dtype=np.float32)
    kqb_rows[1] = np.asarray(inputs["WQ_b"], dtype=np.float32)
    kqv_tail = _bf(np.concatenate([wkq_n, kqb_rows]))
    shared = {
        "wvo": _bf(np.concatenate([wvo_h, extra])),
    }
    key_in = _bf(inputs["key_input"])
    qry_in = _bf(inputs["query_input"])
    val_in = _bf(inputs["value_input"])
    in_maps = []
    for c in range(N_CORES):
        b, qs = c // 2, c % 2
        in_maps.append(dict(
            shared,
            kqv_x=np.ascontiguousarray(np.concatenate([
                key_in[b], qry_in[b, qs * QSH:(qs + 1) * QSH], kqv_tail])),
            vli_x=np.ascontiguousarray(val_in[b]),
        ))
    return in_maps


def _assemble(results):
    out = np.empty((B, T2, DM), dtype=np.float32)
    for c in range(N_CORES):
        b, qs = c // 2, c % 2
        out[b, qs * QSH:(qs + 1) * QSH] = results[c]["out_y"]
    return out


def run_spmd(inputs, **kwargs):
    """Run the kernel on all 8 cores; kwargs forwarded (e.g. trace=True)."""
    nc = _get_nc()
    res = run_bass_kernel_spmd(nc, _make_in_maps(inputs),
                               core_ids=list(range(N_CORES)), **kwargs)
    return res


def kernel(**inputs):
    res = run_spmd(inputs)
    return _assemble(res.results)


# revision 64
# speedup vs baseline: 1.0182x; 1.0019x over previous
"""Multi-head attention Bass/Tile kernel for 8 TRN2 NeuronCores.

Problem: nn_MultiHeadAttention (B=4, T1=T2=2048, d_model=256, d_key=32, H=8,
per-head value dim = d_model).  Reference math (no score scaling, no mask):

    k = key   @ WK^T + bk           [B, T1, 256]   (head h -> cols 32h..32h+32)
    q = query @ WQ^T + bq           [B, T2, 256]
    v = value @ WV^T + bv           [B, T1, 2048]  (head h -> cols 256h..256h+256)
    scores_h = k_h q_h^T            [T1, T2]
    attn = softmax over T1 (keys)
    emb_h = attn^T v_h              [T2, 256]
    out = emb' @ WO^T + bo          emb' channel c = d*8 + h (d outer, h inner)

Sharding: core c handles (batch b = c//2, query half qs = c%2) -> each core
computes the full output slice out[b, qs*1024:(qs+1)*1024, :].  No collectives.

Algebraic restructure (all matmuls bf16, fp32 PSUM):  WV and WO are folded
into per-head G_h[m,o] = sum_d WV[h*256+d, m] WO[o, d*8+h], so the value path
is U_h = val @ G_h (one [2048,256] tensor per head) and the output is
out[q,:] = sum_h (E_h^T U'_h)[q,:]/denom_h[q] + bias, where E = exp(scores),
U' = [U | ones] so PSUM column 256 of the E^T U' matmul IS the softmax
denominator (TRN2 matmul cost scales only with the moving-operand free dim,
so the extra column is free), and bias[o] = wob[o] + sum_h sum_d wvb[h*256+d]
WO[o, d*8+h] (softmax rows sum to 1, so the v-bias is a constant).

Host-side prep (free): everything is cast to bf16 and packed into exactly
TWO dram tensors -- kqv_x = [key; qry; val; WK; WQ; bias rows] feeds one XBAR
DMA-transpose that lands every m-major operand (weight ROWS transpose into
W^T columns, bias rows land as per-partition scalars), and wvo = [WV; WO
head-outer-permuted; v/o bias rows] is one linear DMA.  Per-DMA issue
overhead is ~2.7us and same-queue DMAs serialize, so DMA COUNT, not bytes,
sets the startup latency.  The device does zero layout work on PE/ACT.

The main loop is software-pipelined: scores+exp of iteration i+1 are emitted
before the E^T U' chains of iteration i, so the PE streams scores while ACT
finishes the exps that the E^T U' chains depend on.

kernel(**inputs) takes the FULL unsharded inputs and returns the full output.
"""

import numpy as np
import ml_dtypes
from contextlib import ExitStack

import concourse.bass as bass
import concourse.bacc as bacc
import concourse.mybir as mybir
import concourse.tile as tile
from concourse.bass_utils import run_bass_kernel_spmd

P = 128
B, T1, T2, DM, DK, H = 4, 2048, 2048, 256, 32, 8
QSH = T2 // 2  # queries per core
N_CORES = 8

F32 = mybir.dt.float32
BF16 = mybir.dt.bfloat16
AF = mybir.ActivationFunctionType

ST = T1 // P        # 16 key/seq tiles
QT = QSH // P       # 8 query tiles per core
QC = 512            # query chunk (PSUM free dim)
NQC = QSH // QC     # 2 query chunks
UO = DM + 1         # U columns incl. the ones column (denominator)


def _build_bass():
    nc = bacc.Bacc("TRN2", target_bir_lowering=False, debug=False)

    # kqv = [key; qry; val; WK; WQ] -- one XBAR transpose feeds the whole
    # k/q/v path in m-major layout (weight rows transpose to W^T columns)
    kqv = nc.dram_tensor("kqv_x", [T1 + QSH + 2 * DM + 16, DM], BF16,
                         kind="ExternalInput").ap()
    vli = nc.dram_tensor("vli_x", [T1, DM], BF16, kind="ExternalInput").ap()
    wvo = nc.dram_tensor("wvo", [2 * H * DM + 2 * P, DM], BF16,
                         kind="ExternalInput").ap()
    out = nc.dram_tensor("out_y", [QSH, DM], F32, kind="ExternalOutput").ap()

    with tile.TileContext(nc, pool_alloc_mode="queue") as tc:
        with ExitStack() as ctx:
            _body(ctx, tc, kqv, vli, wvo, out)
    nc.compile()
    return nc


def _body(ctx, tc, kqv, vli, wvo, out):
    nc = tc.nc
    mult, add = mybir.AluOpType.mult, mybir.AluOpType.add
    consts = ctx.enter_context(tc.tile_pool(name="consts", bufs=1))
    main = ctx.enter_context(tc.tile_pool(name="main", bufs=1))
    # One PSUM pool, 3 tags / 8 banks total:
    #   tag S: 2 banks x2      (score tiles [128,2,512] f32)
    #   tag P: 1 bank  x2      (E^T U' output tiles [128,257] f32; bias-const)
    #   tag U: 1 bank  x2      (k/q/U/G projection tiles; warmup)
    pP = ctx.enter_context(tc.tile_pool(name="pP", bufs=1, space="PSUM"))

    bias_bc = consts.tile([P, DM], F32)   # broadcast final bias (filled later)

    # PE warmup: throwaway matmuls on a zeroed tile, overlapping the
    # initial DMAs, so the p-state ramp is done before real matmuls start.
    warm = consts.tile([P, QC], BF16)
    nc.vector.memset(warm, 0.0)
    actwarm = consts.tile([1, 1], BF16)
    nc.scalar.activation(out=actwarm, in_=warm[0:1, 0:1], func=AF.Exp)
    for i in range(20):
        pw = pP.tile([P, QC], F32, tag="U", name=f"warm{i}", bufs=2)
        nc.tensor.matmul(pw, warm[:, 0:P], warm, start=True, stop=True)

    # persistent bf16 tensors
    kT = main.tile([P, 2, T1], BF16)      # [c, s]
    qT = main.tile([P, 2, QSH], BF16)     # [c, q]
    kqvT = main.tile([P, 2, T1 + QSH + 2 * DM + 16], BF16)
    valT = main.tile([P, 2, T1], BF16)
    Gt = main.tile([P, 2, H, DM], BF16)   # [m, mt, h, o]
    uT = main.tile([P, 2, ST, UO], BF16)  # [s, hslot, st, o]; col 256 = 1.0
    acc = main.tile([P, QT, DM], F32)     # output accumulator [q, cout]
    nc.vector.memset(uT[:, :, :, DM:UO], 1.0)

    # ---------------- stage 0: DMA loads/transposes + projections -----------
    with ExitStack() as s0:
        stg = s0.enter_context(tc.tile_pool(name="stg", bufs=1))

        # Minimal DMA count: per-DMA issue overhead is ~2.7us and queue DMAs
        # serialize, so key/qry/val ride ONE stacked XBAR transpose.
        nc.sync.dma_start_transpose(kqvT, kqv)
        wvo_bf = main.tile([P, 2 * ST + 2, DM], BF16)
        nc.sync.dma_start(out=wvo_bf, in_=wvo.rearrange("(t p) d -> p t d", p=P))
        nc.sync.dma_start_transpose(valT, vli)
        nb = T1 + QSH + 2 * DM
        kqb_f = consts.tile([P, 2, 2], F32)   # f32 scalars for tensor_scalar
        nc.vector.tensor_copy(out=kqb_f, in_=kqvT[:, :, nb:nb + 2])
        wk_b, wq_b = kqb_f[:, :, 0:1], kqb_f[:, :, 1:2]
        wvb_bf = wvo_bf[:, 2 * ST, 4:4 + ST]
        wob_f = wvo_bf[0:1, 2 * ST + 1, :]
        keyT = kqvT[:, :, 0:T1]               # [m, s]
        qryT = kqvT[:, :, T1:T1 + QSH]        # [m, q]
        wkT = kqvT[:, :, T1 + QSH:T1 + QSH + DM]              # [m, c]
        wqT = kqvT[:, :, T1 + QSH + DM:T1 + QSH + 2 * DM]
        wv_bf = wvo_bf[:, 0:ST, :]            # [c_v, kt, m] (natural)
        woTp = wvo_bf[:, ST:2 * ST, :]        # [d (in-head), kt=2h+db, o]

        # k/q projections: kT[c, s] = sum_m wkT[m, c] keyT[m, s]  (+bias)
        for ct in range(2):
            for sc in range(T1 // 512):
                pp = pP.tile([P, 512], F32, tag="U", name=f"ppk{ct}_{sc}", bufs=2)
                for dt in range(2):
                    nc.tensor.matmul(pp, wkT[:, dt, ct * P:(ct + 1) * P],
                                     keyT[:, dt, sc * 512:(sc + 1) * 512],
                                     start=(dt == 0), stop=(dt == 1))
                nc.vector.tensor_scalar(out=kT[:, ct, sc * 512:(sc + 1) * 512],
                                        in0=pp, scalar1=wk_b[:, ct, 0:1],
                                        scalar2=None, op0=add)
            for sc in range(QSH // 512):
                pp = pP.tile([P, 512], F32, tag="U", name=f"ppq{ct}_{sc}", bufs=2)
                for dt in range(2):
                    nc.tensor.matmul(pp, wqT[:, dt, ct * P:(ct + 1) * P],
                                     qryT[:, dt, sc * 512:(sc + 1) * 512],
                                     start=(dt == 0), stop=(dt == 1))
                nc.vector.tensor_scalar(out=qT[:, ct, sc * 512:(sc + 1) * 512],
                                        in0=pp, scalar1=wq_b[:, ct, 0:1],
                                        scalar2=None, op0=add)

        # bias_bc[o] = wob[o] + sum_h sum_d wvb[h*256+d] WO[o, d*8+h]
        pb = pP.tile([1, DM], F32, tag="P", name="pbias", bufs=2)
        for kt in range(ST):
            nc.tensor.matmul(pb, wvb_bf[:, kt:kt + 1], woTp[:, kt, :],
                             start=(kt == 0), stop=(kt == ST - 1))
        bias1 = consts.tile([1, DM], F32)
        nc.vector.tensor_add(bias1, pb, wob_f)
        nc.gpsimd.partition_broadcast(bias_bc, bias1)

    # ---------------- main loop: one head at a time, software-pipelined -----
    with ExitStack() as sm:
        sE = sm.enter_context(tc.tile_pool(name="sE", bufs=2))
        ssm = sm.enter_context(tc.tile_pool(name="ssm", bufs=4))

        out_r = out.rearrange("(n p) d -> p n d", p=P)

        def emit_po(h, qc, E):
            """out_h[q, :] = E^T U' (col 256 = denominator), normalize, acc.
            On the last head, stream each finished acc tile straight out."""
            hs = h % 2
            for qt in range(QC // P):
                po = pP.tile([P, UO], F32, tag="P",
                             name=f"po{h}_{qc}_{qt}", bufs=2)
                for st in range(ST):
                    nc.tensor.matmul(po, E[:, st, qt * P:(qt + 1) * P],
                                     uT[:, hs, st, :],
                                     start=(st == 0), stop=(st == ST - 1))
                rc = ssm.tile([P, 1], F32, tag="rc", name=f"rc{h}_{qc}_{qt}")
                nc.vector.reciprocal(out=rc, in_=po[:, DM:UO])
                gqt = qc * (QC // P) + qt
                nc.vector.scalar_tensor_tensor(
                    out=acc[:, gqt, :], in0=po[:, 0:DM], scalar=rc,
                    in1=(bias_bc if h == 0 else acc[:, gqt, :]),
                    op0=mult, op1=add)
                if h == H - 1 and (qc == 0 and qt % 2 == 1 or qc == 1):
                    g0 = gqt - 1 if qc == 0 else gqt
                    n = 2 if qc == 0 else 1
                    nc.sync.dma_start(out=out_r[:, g0:g0 + n, :],
                                      in_=acc[:, g0:g0 + n, :])

        def emit_u(h):
            """G_h = WV_h/WO_h fold, then U_h[s, o] = sum_m val[s, m] G_h[m, o];
            col 256 stays 1.0.  G rides inside the main loop so its PSUM->SBUF
            copies hide behind main-loop DVE slack instead of stalling stage 0."""
            hs = h % 2
            pg = pP.tile([P, 2, DM], F32, tag="U", name=f"pg{h}", bufs=2)
            for mt in range(2):
                for db in range(2):
                    nc.tensor.matmul(pg[:, mt, :],
                                     wv_bf[:, 2 * h + db, mt * P:(mt + 1) * P],
                                     woTp[:, 2 * h + db, :],
                                     start=(db == 0), stop=(db == 1))
            nc.vector.tensor_copy(out=Gt[:, :, h, :], in_=pg)
            for sp in range(ST // 2):
                pu = pP.tile([P, 2, DM], F32, tag="U", name=f"pu{h}_{sp}", bufs=2)
                for i in range(2):
                    st = 2 * sp + i
                    for mt in range(2):
                        nc.tensor.matmul(pu[:, i, :],
                                         valT[:, mt, st * P:(st + 1) * P],
                                         Gt[:, mt, h, :],
                                         start=(mt == 0), stop=(mt == 1))
                nc.vector.tensor_copy(out=uT[:, hs, 2 * sp:2 * sp + 2, 0:DM],
                                      in_=pu)

        prev = None
        for h in range(H):
            if h > 0:
                emit_u(h)
            base, ctile = 32 * (h % 4), h // 4
            for qc in range(NQC):
                E = sE.tile([P, ST, QC], BF16, tag="E", name=f"E{h}_{qc}")
                # phase 1: scores + exp.  scores_h[s, q] = kT_h^T qT_h
                for sp in range(ST // 2):
                    ps = pP.tile([P, 2, QC], F32, tag="S",
                                 name=f"sc{h}_{qc}_{sp}", bufs=2)
                    for i in range(2):
                        st = 2 * sp + i
                        nc.tensor.matmul(
                            ps[:, i, :],
                            kT[base:base + 32, ctile, st * P:(st + 1) * P],
                            qT[base:base + 32, ctile, qc * QC:(qc + 1) * QC],
                            start=True, stop=True, tile_position=(base, 0))
                    nc.scalar.activation(out=E[:, 2 * sp:2 * sp + 2, :], in_=ps,
                                         func=AF.Exp)
                if h == 0 and qc == 0:
                    emit_u(0)
                if prev is not None:
                    emit_po(*prev)
                prev = (h, qc, E)
        emit_po(*prev)


_NC_CACHE = None


def _get_nc():
    global _NC_CACHE
    if _NC_CACHE is None:
        _NC_CACHE = _build_bass()
    return _NC_CACHE


def _bf(x):
    return np.ascontiguousarray(np.asarray(x, dtype=np.float32).astype(
        ml_dtypes.bfloat16))


def _make_in_maps(inputs):
    wo = np.asarray(inputs["WO_w"], dtype=np.float32)     # [256, 2048]
    # woTp row (2h+db)*128+d' = WO[:, (db*128+d')*8+h]
    wotp = wo.reshape(DM, 2, P, H).transpose(3, 1, 2, 0).reshape(H * DM, DM)
    wvo_h = np.concatenate([np.asarray(inputs["WV_w"], dtype=np.float32), wotp])
    # wvo tail rows: row +0 cols 4:20 = WV_b (column kt = partition slice of
    # it), row +1 = WO_b
    extra = np.zeros((2 * P, DM), dtype=np.float32)
    extra[0:P, 4:4 + ST] = np.asarray(
        inputs["WV_b"], dtype=np.float32).reshape(ST, P).T
    extra[P, :] = np.asarray(inputs["WO_b"], dtype=np.float32)
    # kqv tail: WK/WQ rows (transpose to W^T), then 16 pad rows whose first
    # two are WK_b/WQ_b (a transposed bias row lands as [p, tile] scalars)
    wkq_n = np.concatenate([np.asarray(inputs["WK_w"], dtype=np.float32),
                            np.asarray(inputs["WQ_w"], dtype=np.float32)])
    kqb_rows = np.zeros((16, DM), dtype=np.float32)
    kqb_rows[0] = np.asarray(inputs["WK_b"], dtype=np.float32)
    kqb_rows[1] = np.asarray(inputs["WQ_b"], dtype=np.float32)
    kqv_tail = _bf(np.concatenate([wkq_n, kqb_rows]))
    shared = {
        "wvo": _bf(np.concatenate([wvo_h, extra])),
    }
    key_in = _bf(inputs["key_input"])
    qry_in = _bf(inputs["query_input"])
    val_in = _bf(inputs["value_input"])
    in_maps = []
    for c in range(N_CORES):
        b, qs = c // 2, c % 2
        in_maps.append(dict(
            shared,
            kqv_x=np.ascontiguousarray(np.concatenate([
                key_in[b], qry_in[b, qs * QSH:(qs + 1) * QSH], kqv_tail])),
            vli_x=np.ascontiguousarray(val_in[b]),
        ))
    return in_maps


def _assemble(results):
    out = np.empty((B, T2, DM), dtype=np.float32)
    for c in range(N_CORES):
        b, qs = c // 2, c % 2
        out[b, qs * QSH:(qs + 1) * QSH] = results[c]["out_y"]
    return out


def run_spmd(inputs, **kwargs):
    """Run the kernel on all 8 cores; kwargs forwarded (e.g. trace=True)."""
    nc = _get_nc()
    res = run_bass_kernel_spmd(nc, _make_in_maps(inputs),
                               core_ids=list(range(N_CORES)), **kwargs)
    return res


def kernel(**inputs):
    res = run_spmd(inputs)
    return _assemble(res.results)


# revision 68
# speedup vs baseline: 1.0387x; 1.0202x over previous
"""Multi-head attention Bass/Tile kernel for 8 TRN2 NeuronCores.

Problem: nn_MultiHeadAttention (B=4, T1=T2=2048, d_model=256, d_key=32, H=8,
per-head value dim = d_model).  Reference math (no score scaling, no mask):

    k = key   @ WK^T + bk           [B, T1, 256]   (head h -> cols 32h..32h+32)
    q = query @ WQ^T + bq           [B, T2, 256]
    v = value @ WV^T + bv           [B, T1, 2048]  (head h -> cols 256h..256h+256)
    scores_h = k_h q_h^T            [T1, T2]
    attn = softmax over T1 (keys)
    emb_h = attn^T v_h              [T2, 256]
    out = emb' @ WO^T + bo          emb' channel c = d*8 + h (d outer, h inner)

Sharding: core c handles (batch b = c//2, query half qs = c%2) -> each core
computes the full output slice out[b, qs*1024:(qs+1)*1024, :].  No collectives.

Algebraic restructure (all matmuls bf16, fp32 PSUM):  WV and WO are folded
into per-head G_h[m,o] = sum_d WV[h*256+d, m] WO[o, d*8+h], so the value path
is U_h = val @ G_h (one [2048,256] tensor per head) and the output is
out[q,:] = sum_h (E_h^T U'_h)[q,:]/denom_h[q] + bias, where E = exp(scores),
U' = [U | ones] so PSUM column 256 of the E^T U' matmul IS the softmax
denominator (TRN2 matmul cost scales only with the moving-operand free dim,
so the extra column is free), and bias[o] = wob[o] + sum_h sum_d wvb[h*256+d]
WO[o, d*8+h] (softmax rows sum to 1, so the v-bias is a constant).

Host-side prep (free): everything is cast to bf16 and packed into exactly
TWO dram tensors -- kqv_x = [key; qry; val; WK; WQ; bias rows] feeds one XBAR
DMA-transpose that lands every m-major operand (weight ROWS transpose into
W^T columns, bias rows land as per-partition scalars), and wvo = [WV; WO
head-outer-permuted; v/o bias rows] is one linear DMA.  Per-DMA issue
overhead is ~2.7us and same-queue DMAs serialize, so DMA COUNT, not bytes,
sets the startup latency.  The device does zero layout work on PE/ACT.

The main loop is software-pipelined: scores+exp of iteration i+1 are emitted
before the E^T U' chains of iteration i, so the PE streams scores while ACT
finishes the exps that the E^T U' chains depend on.

kernel(**inputs) takes the FULL unsharded inputs and returns the full output.
"""

import numpy as np
import ml_dtypes
from contextlib import ExitStack

import concourse.bass as bass
import concourse.bacc as bacc
import concourse.mybir as mybir
import concourse.tile as tile
from concourse.bass_utils import run_bass_kernel_spmd
from concourse.masks import make_identity

P = 128
B, T1, T2, DM, DK, H = 4, 2048, 2048, 256, 32, 8
QSH = T2 // 2  # queries per core
N_CORES = 8

F32 = mybir.dt.float32
BF16 = mybir.dt.bfloat16
AF = mybir.ActivationFunctionType

ST = T1 // P        # 16 key/seq tiles
QT = QSH // P       # 8 query tiles per core
QC = 512            # query chunk (PSUM free dim)
NQC = QSH // QC     # 2 query chunks
UO = DM + 1         # U columns incl. the ones column (denominator)


def _build_bass():
    nc = bacc.Bacc("TRN2", target_bir_lowering=False, debug=False)

    # kqv = [key; qry; val; WK; WQ] -- one XBAR transpose feeds the whole
    # k/q/v path in m-major layout (weight rows transpose to W^T columns)
    kqv = nc.dram_tensor("kqv_x", [T1 + QSH + 2 * DM + 16, DM], BF16,
                         kind="ExternalInput").ap()
    vli = nc.dram_tensor("vli_x", [T1, DM], BF16, kind="ExternalInput").ap()
    wvo = nc.dram_tensor("wvo", [2 * H * DM + 2 * P, DM], BF16,
                         kind="ExternalInput").ap()
    out = nc.dram_tensor("out_y", [QSH, DM], F32, kind="ExternalOutput").ap()

    with tile.TileContext(nc, pool_alloc_mode="queue") as tc:
        with ExitStack() as ctx:
            _body(ctx, tc, kqv, vli, wvo, out)
    nc.compile()
    return nc


def _body(ctx, tc, kqv, vli, wvo, out):
    nc = tc.nc
    mult, add = mybir.AluOpType.mult, mybir.AluOpType.add
    consts = ctx.enter_context(tc.tile_pool(name="consts", bufs=1))
    main = ctx.enter_context(tc.tile_pool(name="main", bufs=1))
    # One PSUM pool, 3 tags / 8 banks total:
    #   tag S: 2 banks x2      (score tiles [128,2,512] f32)
    #   tag P: 1 bank  x2      (E^T U' output tiles [128,257] f32; bias-const)
    #   tag U: 1 bank  x2      (k/q/U/G projection tiles; warmup)
    pP = ctx.enter_context(tc.tile_pool(name="pP", bufs=1, space="PSUM"))

    bias_bc = consts.tile([P, DM], F32)   # broadcast final bias (filled later)
    ident_bf = consts.tile([P, P], BF16)
    make_identity(nc, ident_bf)

    # PE warmup: throwaway matmuls on a zeroed tile, overlapping the
    # initial DMAs, so the p-state ramp is done before real matmuls start.
    warm = consts.tile([P, QC], BF16)
    nc.vector.memset(warm, 0.0)
    actwarm = consts.tile([1, 1], BF16)
    nc.scalar.activation(out=actwarm, in_=warm[0:1, 0:1], func=AF.Exp)
    for i in range(20):
        pw = pP.tile([P, QC], F32, tag="U", name=f"warm{i}", bufs=4)
        nc.tensor.matmul(pw, warm[:, 0:P], warm, start=True, stop=True)

    # persistent bf16 tensors
    kT = main.tile([P, 2, T1], BF16)      # [c, s]
    qT = main.tile([P, 2, QSH], BF16)     # [c, q]
    kqvT = main.tile([P, 2, T1 + QSH + 2 * DM + 16], BF16)
    Gt = main.tile([P, 2, H, DM], BF16)   # [m, mt, h, o]
    vA = main.tile([P, ST, UO], BF16)     # val_aug [s, st, m]; col 256 = 1.0
    acc = main.tile([P, QT, DM], F32)     # output accumulator [q, cout]
    nc.vector.memset(vA[:, :, DM:UO], 1.0)

    # ---------------- stage 0: DMA loads/transposes + projections -----------
    with ExitStack() as s0:
        stg = s0.enter_context(tc.tile_pool(name="stg", bufs=1))

        # Minimal DMA count: per-DMA issue overhead is ~2.7us and queue DMAs
        # serialize, so key/qry/val ride ONE stacked XBAR transpose.
        nc.sync.dma_start_transpose(kqvT, kqv)
        nc.sync.dma_start(out=vA[:, :, 0:DM],
                          in_=vli.rearrange("(t p) d -> p t d", p=P))
        wvo_bf = main.tile([P, 2 * ST + 2, DM], BF16)
        nc.sync.dma_start(out=wvo_bf, in_=wvo.rearrange("(t p) d -> p t d", p=P))
        nb = T1 + QSH + 2 * DM
        kqb_f = consts.tile([P, 2, 2], F32)   # f32 scalars for tensor_scalar
        nc.vector.tensor_copy(out=kqb_f, in_=kqvT[:, :, nb:nb + 2])
        wk_b, wq_b = kqb_f[:, :, 0:1], kqb_f[:, :, 1:2]
        wvb_bf = wvo_bf[:, 2 * ST, 4:4 + ST]
        wob_f = wvo_bf[0:1, 2 * ST + 1, :]
        keyT = kqvT[:, :, 0:T1]               # [m, s]
        qryT = kqvT[:, :, T1:T1 + QSH]        # [m, q]
        wkT = kqvT[:, :, T1 + QSH:T1 + QSH + DM]              # [m, c]
        wqT = kqvT[:, :, T1 + QSH + DM:T1 + QSH + 2 * DM]
        wv_bf = wvo_bf[:, 0:ST, :]            # [c_v, kt, m] (natural)
        woTp = wvo_bf[:, ST:2 * ST, :]        # [d (in-head), kt=2h+db, o]

        # k/q projections: kT[c, s] = sum_m wkT[m, c] keyT[m, s]  (+bias)
        for ct in range(2):
            for sc in range(T1 // 512):
                pp = pP.tile([P, 512], F32, tag="U", name=f"ppk{ct}_{sc}", bufs=4)
                for dt in range(2):
                    nc.tensor.matmul(pp, wkT[:, dt, ct * P:(ct + 1) * P],
                                     keyT[:, dt, sc * 512:(sc + 1) * 512],
                                     start=(dt == 0), stop=(dt == 1))
                nc.vector.tensor_scalar(out=kT[:, ct, sc * 512:(sc + 1) * 512],
                                        in0=pp, scalar1=wk_b[:, ct, 0:1],
                                        scalar2=None, op0=add)
            for sc in range(QSH // 512):
                pp = pP.tile([P, 512], F32, tag="U", name=f"ppq{ct}_{sc}", bufs=4)
                for dt in range(2):
                    nc.tensor.matmul(pp, wqT[:, dt, ct * P:(ct + 1) * P],
                                     qryT[:, dt, sc * 512:(sc + 1) * 512],
                                     start=(dt == 0), stop=(dt == 1))
                nc.vector.tensor_scalar(out=qT[:, ct, sc * 512:(sc + 1) * 512],
                                        in0=pp, scalar1=wq_b[:, ct, 0:1],
                                        scalar2=None, op0=add)

        # bias_bc[o] = wob[o] + sum_h sum_d wvb[h*256+d] WO[o, d*8+h]
        pb = pP.tile([1, DM], F32, tag="P", name="pbias", bufs=2)
        for kt in range(ST):
            nc.tensor.matmul(pb, wvb_bf[:, kt:kt + 1], woTp[:, kt, :],
                             start=(kt == 0), stop=(kt == ST - 1))
        bias1 = consts.tile([1, DM], F32)
        nc.vector.tensor_add(bias1, pb, wob_f)
        nc.gpsimd.partition_broadcast(bias_bc, bias1)

    # ---------------- main loop: one head at a time, software-pipelined -----
    with ExitStack() as sm:
        sE = sm.enter_context(tc.tile_pool(name="sE", bufs=2))
        ssm = sm.enter_context(tc.tile_pool(name="ssm", bufs=4))

        out_r = out.rearrange("(n p) d -> p n d", p=P)

        pend = []   # finalize-work FIFO: (h, gqt, ctxn)

        def emit_fin():
            """Pop one pending (h, qt): transpose normalized ctx to m-major,
            multiply by G_h, accumulate into acc (+bias on h=0), stream out
            on the last head."""
            h, gqt, ctxn = pend.pop(0)
            ctxT = ssm.tile([P, 2, P], BF16, tag="ctxT", name=f"cT{h}_{gqt}",
                            bufs=3)
            ptp = pP.tile([P, 2, P], BF16, tag="U", name=f"tp{h}_{gqt}", bufs=4)
            for mt in range(2):
                nc.tensor.transpose(ptp[:, mt, :], ctxn[:, mt * P:(mt + 1) * P],
                                    ident_bf)
            nc.vector.tensor_copy(out=ctxT, in_=ptp)
            og = pP.tile([P, DM], F32, tag="U", name=f"og{h}_{gqt}", bufs=4)
            for mt in range(2):
                nc.tensor.matmul(og, ctxT[:, mt, :], Gt[:, mt, h, :],
                                 start=(mt == 0), stop=(mt == 1))
            nc.vector.tensor_tensor(
                out=acc[:, gqt, :], in0=og,
                in1=(bias_bc if h == 0 else acc[:, gqt, :]),
                op=mybir.AluOpType.add)
            if h == H - 1:
                nc.sync.dma_start(out=out_r[:, gqt:gqt + 1, :],
                                  in_=acc[:, gqt:gqt + 1, :])

        def emit_ctx(h, qc, E):
            """ctx_h[q, :] = E^T [val | 1] (col 256 = denominator), normalize
            to bf16; the G_h application is queued on the finalize FIFO."""
            for qt in range(QC // P):
                po = pP.tile([P, UO], F32, tag="P",
                             name=f"po{h}_{qc}_{qt}", bufs=2)
                for st in range(ST):
                    nc.tensor.matmul(po, E[:, st, qt * P:(qt + 1) * P],
                                     vA[:, st, :],
                                     start=(st == 0), stop=(st == ST - 1))
                rc = ssm.tile([P, 1], F32, tag="rc", name=f"rc{h}_{qc}_{qt}")
                nc.vector.reciprocal(out=rc, in_=po[:, DM:UO])
                gqt = qc * (QC // P) + qt
                ctxn = ssm.tile([P, DM], BF16, tag="ctxn",
                                name=f"cn{h}_{qc}_{qt}", bufs=3)
                nc.vector.tensor_scalar(out=ctxn, in0=po[:, 0:DM], scalar1=rc,
                                        scalar2=None, op0=mult)
                pend.append((h, gqt, ctxn))
                if len(pend) > 2:
                    emit_fin()

        def emit_u(h):
            """G_h = WV_h/WO_h fold (inside the loop: its PSUM->SBUF copy
            hides behind main-loop DVE slack instead of stalling stage 0)."""
            pg = pP.tile([P, 2, DM], F32, tag="U", name=f"pg{h}", bufs=4)
            for mt in range(2):
                for db in range(2):
                    nc.tensor.matmul(pg[:, mt, :],
                                     wv_bf[:, 2 * h + db, mt * P:(mt + 1) * P],
                                     woTp[:, 2 * h + db, :],
                                     start=(db == 0), stop=(db == 1))
            nc.vector.tensor_copy(out=Gt[:, :, h, :], in_=pg)

        prev = None
        for h in range(H):
            if h > 0:
                emit_u(h)
            base, ctile = 32 * (h % 4), h // 4
            for qc in range(NQC):
                E = sE.tile([P, ST, QC], BF16, tag="E", name=f"E{h}_{qc}")
                # phase 1: scores + exp.  scores_h[s, q] = kT_h^T qT_h
                for st in range(ST):
                    ps = pP.tile([P, QC], F32, tag="S",
                                 name=f"sc{h}_{qc}_{st}", bufs=2)
                    nc.tensor.matmul(
                        ps,
                        kT[base:base + 32, ctile, st * P:(st + 1) * P],
                        qT[base:base + 32, ctile, qc * QC:(qc + 1) * QC],
                        start=True, stop=True, tile_position=(base, 0))
                    nc.scalar.activation(out=E[:, st, :], in_=ps, func=AF.Exp)
                if h == 0 and qc == 0:
                    emit_u(0)
                if prev is not None:
                    emit_ctx(*prev)
                prev = (h, qc, E)
        emit_ctx(*prev)
        while pend:
            emit_fin()


_NC_CACHE = None


def _get_nc():
    global _NC_CACHE
    if _NC_CACHE is None:
        _NC_CACHE = _build_bass()
    return _NC_CACHE


def _bf(x):
    return np.ascontiguousarray(np.asarray(x, dtype=np.float32).astype(
        ml_dtypes.bfloat16))


def _make_in_maps(inputs):
    wo = np.asarray(inputs["WO_w"], dtype=np.float32)     # [256, 2048]
    # woTp row (2h+db)*128+d' = WO[:, (db*128+d')*8+h]
    wotp = wo.reshape(DM, 2, P, H).transpose(3, 1, 2, 0).reshape(H * DM, DM)
    wvo_h = np.concatenate([np.asarray(inputs["WV_w"], dtype=np.float32), wotp])
    # wvo tail rows: row +0 cols 4:20 = WV_b (column kt = partition slice of
    # it), row +1 = WO_b
    extra = np.zeros((2 * P, DM), dtype=np.float32)
    extra[0:P, 4:4 + ST] = np.asarray(
        inputs["WV_b"], dtype=np.float32).reshape(ST, P).T
    extra[P, :] = np.asarray(inputs["WO_b"], dtype=np.float32)
    # kqv tail: WK/WQ rows (transpose to W^T), then 16 pad rows whose first
    # two are WK_b/WQ_b (a transposed bias row lands as [p, tile] scalars)
    wkq_n = np.concatenate([np.asarray(inputs["WK_w"], dtype=np.float32),
                            np.asarray(inputs["WQ_w"], dtype=np.float32)])
    kqb_rows = np.zeros((16, DM), dtype=np.float32)
    kqb_rows[0] = np.asarray(inputs["WK_b"], dtype=np.float32)
    kqb_rows[1] = np.asarray(inputs["WQ_b"], dtype=np.float32)
    kqv_tail = _bf(np.concatenate([wkq_n, kqb_rows]))
    shared = {
        "wvo": _bf(np.concatenate([wvo_h, extra])),
    }
    key_in = _bf(inputs["key_input"])
    qry_in = _bf(inputs["query_input"])
    val_in = _bf(inputs["value_input"])
    in_maps = []
    for c in range(N_CORES):
        b, qs = c // 2, c % 2
        in_maps.append(dict(
            shared,
            kqv_x=np.ascontiguousarray(np.concatenate([
                key_in[b], qry_in[b, qs * QSH:(qs + 1) * QSH], kqv_tail])),
            vli_x=np.ascontiguousarray(val_in[b]),
        ))
    return in_maps


def _assemble(results):
    out = np.empty((B, T2, DM), dtype=np.float32)
    for c in range(N_CORES):
        b, qs = c // 2, c % 2
        out[b, qs * QSH:(qs + 1) * QSH] = results[c]["out_y"]
    return out


def run_spmd(inputs, **kwargs):
    """Run the kernel on all 8 cores; kwargs forwarded (e.g. trace=True)."""
    nc = _get_nc()
    res = run_bass_kernel_spmd(nc, _make_in_maps(inputs),
                               core_ids=list(range(N_CORES)), **kwargs)
    return res


def kernel(**inputs):
    res = run_spmd(inputs)
    return _assemble(res.results)
